# revision 10
# baseline (speedup 1.0000x reference)
"""MeanAggregator (GNN segment-mean) Bass kernel for 8 Trainium2 NeuronCores.

Reference computation:
    gathered = features[edge_dst]                       # [E, D]
    sums     = segment_sum(gathered, edge_seg, B)       # [B, D]
    counts   = segment_sum(ones(E), edge_seg, B)        # [B]
    out      = sums / counts[:, None]                   # [B, D]

Strategy: shard output nodes (segments) contiguously across the 8 cores.
edge_seg is sorted, so each core's edges are a contiguous slice and each
core owns its output rows outright -- no collectives.  Per 128-node tile,
ONE indirect DMA gathers all K neighbor rows per node into a
[128, K*128] SBUF tile (the offset AP carries K indices per partition),
then a pairwise tree of VectorE adds reduces the K blocks and the
ScalarE scales by 1/K on the way out.
"""

import sys

for _p in ("/opt/trn_rl_repo", "/root/.axon_site/_ro/trn_rl_repo"):
    if _p not in sys.path:
        sys.path.append(_p)

import numpy as np

from concourse import bacc, bass, mybir
import concourse.tile as tile
from concourse.bass_utils import run_bass_kernel_spmd

TRACE = False            # set by test.py to profile the HW run
TRACE_KWARGS = {"trace": True}
LAST_RESULT = None

P = 128          # SBUF partitions = nodes per tile
D = 128          # feature dim
N_CORES = 8
N_TOTAL = 100000  # feature table rows


def build_program(n_tiles: int, K: int, weighted: bool,
                  g_bufs: int = 3, repeats: int = 1) -> bass.Bass:
    """Bass program run identically on every core.

    Inputs per core:
      features [N_TOTAL, D] f32  (replicated)
      idx      [n_tiles*P, K] i32  (this core's neighbor ids, padded)
      wts      [n_tiles*P, K] f32  (only if weighted: per-edge weight, e.g.
                                    1/count with 0 for padding)
    Output per core:
      out      [n_tiles*P, D] f32
    """
    nodes = n_tiles * P
    nc = bacc.Bacc("TRN2", target_bir_lowering=False)
    feat = nc.declare_dram_parameter("features", [N_TOTAL, D],
                                     mybir.dt.float32, isOutput=False)
    idx = nc.declare_dram_parameter("idx", [nodes, K],
                                    mybir.dt.int32, isOutput=False)
    if weighted:
        wts = nc.declare_dram_parameter("wts", [nodes, K],
                                        mybir.dt.float32, isOutput=False)
    out = nc.declare_dram_parameter("out", [nodes, D],
                                    mybir.dt.float32, isOutput=True)

    with tile.TileContext(nc) as tc:
        with tc.tile_pool(name="gath", bufs=g_bufs) as gp, \
             tc.tile_pool(name="io", bufs=4) as iop, \
             tc.tile_pool(name="res", bufs=4) as rp:
            for t in range(n_tiles * repeats):
                t = t % n_tiles
                sl = slice(t * P, (t + 1) * P)
                idx_t = iop.tile([P, K], mybir.dt.int32, tag="idx")
                nc.sync.dma_start(out=idx_t[:], in_=idx[sl, :])
                G = gp.tile([P, K * D], mybir.dt.float32, tag="g")
                # NOTE: one indirect DMA per neighbor column. A single DMA
                # with a [P, K] offset AP is NOT equivalent on real HW (the
                # DGE scrambles multi-index-per-partition gathers).
                for j in range(K):
                    nc.gpsimd.indirect_dma_start(
                        out=G[:, j * D:(j + 1) * D],
                        out_offset=None,
                        in_=feat[:],
                        in_offset=bass.IndirectOffsetOnAxis(
                            ap=idx_t[:, j:j + 1], axis=0),
                    )
                if weighted:
                    w_t = iop.tile([P, K], mybir.dt.float32, tag="w")
                    nc.sync.dma_start(out=w_t[:], in_=wts[sl, :])
                    for j in range(K):
                        nc.vector.tensor_scalar_mul(
                            out=G[:, j * D:(j + 1) * D],
                            in0=G[:, j * D:(j + 1) * D],
                            scalar1=w_t[:, j:j + 1],
                        )
                # pairwise in-place tree reduction of the K blocks of G
                cur = K
                while cur > 1:
                    h = cur // 2
                    nc.vector.tensor_tensor(
                        out=G[:, :h * D],
                        in0=G[:, :h * D],
                        in1=G[:, h * D:2 * h * D],
                        op=mybir.AluOpType.add,
                    )
                    if cur % 2:
                        nc.vector.tensor_tensor(
                            out=G[:, (h - 1) * D:h * D],
                            in0=G[:, (h - 1) * D:h * D],
                            in1=G[:, (cur - 1) * D:cur * D],
                            op=mybir.AluOpType.add,
                        )
                    cur = h
                o_t = rp.tile([P, D], mybir.dt.float32, tag="o")
                if weighted:
                    # weights already include the 1/count factor
                    nc.vector.tensor_copy(out=o_t[:], in_=G[:, :D])
                else:
                    nc.vector.tensor_scalar_mul(out=o_t[:], in0=G[:, :D],
                                                scalar1=1.0 / K)
                nc.sync.dma_start(out=out[sl, :], in_=o_t[:])
    nc.compile()
    return nc


def _prep_general(edge_seg, edge_dst, B):
    """Arbitrary sorted-or-not edge_seg: build padded [B, K] index and
    weight matrices (weight = 1/count, 0 on padding)."""
    E = edge_dst.shape[0]
    order = np.argsort(edge_seg, kind="stable")
    sseg = edge_seg[order].astype(np.int64)
    sdst = edge_dst[order].astype(np.int32)
    counts = np.bincount(sseg, minlength=B).astype(np.int64)
    K = max(int(counts.max()), 1) if E else 1
    starts = np.zeros(B, np.int64)
    np.cumsum(counts[:-1], out=starts[1:])
    pos = np.arange(E, dtype=np.int64) - np.repeat(starts, counts)
    idx_mat = np.zeros((B, K), np.int32)
    wts_mat = np.zeros((B, K), np.float32)
    idx_mat[sseg, pos] = sdst
    inv = np.zeros(B, np.float32)
    nz = counts > 0
    inv[nz] = 1.0 / counts[nz]
    wts_mat[sseg, pos] = inv[sseg]
    return idx_mat, wts_mat, K


def kernel(features, edge_seg, edge_dst, num_nodes=None, **_unused):
    features = np.ascontiguousarray(np.asarray(features, dtype=np.float32))
    edge_seg = np.asarray(edge_seg)
    edge_dst = np.asarray(edge_dst)
    E = int(edge_dst.shape[0])
    if num_nodes is not None:
        B = int(np.asarray(num_nodes))
    else:
        B = int(edge_seg.max()) + 1

    # Fast path: canonical uniform-degree layout (reference's setup_inputs):
    # edge_seg == repeat(arange(B), K) -> just reshape edge_dst.
    K = E // B if B and E % B == 0 else 0
    uniform = K > 0 and np.array_equal(
        edge_seg, np.repeat(np.arange(B, dtype=edge_seg.dtype), K))
    if uniform:
        idx_mat = np.ascontiguousarray(edge_dst.reshape(B, K).astype(np.int32))
        wts_mat = None
    else:
        idx_mat, wts_mat, K = _prep_general(edge_seg, edge_dst, B)

    # Shard rows contiguously across cores; pad each shard to a tile multiple.
    npc = -(-B // N_CORES)           # nodes per core (ceil)
    n_tiles = -(-npc // P)
    nodes_pad = n_tiles * P
    weighted = wts_mat is not None

    nc = build_program(n_tiles, K, weighted)

    in_maps = []
    for c in range(N_CORES):
        lo = c * npc
        hi = min(B, (c + 1) * npc)
        idx_c = np.zeros((nodes_pad, K), np.int32)
        if hi > lo:
            idx_c[:hi - lo] = idx_mat[lo:hi]
        m = {"features": features, "idx": idx_c}
        if weighted:
            w_c = np.zeros((nodes_pad, K), np.float32)
            if hi > lo:
                w_c[:hi - lo] = wts_mat[lo:hi]
            m["wts"] = w_c
        in_maps.append(m)

    kw = dict(TRACE_KWARGS) if TRACE else {}
    res = run_bass_kernel_spmd(nc, in_maps, list(range(N_CORES)), **kw)
    global LAST_RESULT
    LAST_RESULT = res
    parts = []
    for c in range(N_CORES):
        lo = c * npc
        hi = min(B, (c + 1) * npc)
        if hi > lo:
            parts.append(res.results[c]["out"][:hi - lo])
    return np.concatenate(parts, axis=0)
